# revision 23
# baseline (speedup 1.0000x reference)
"""Depthwise causal-conv1d step (single timestep) on 8 Trainium2 cores.

  out[b, h]         = sum_k w[h, k] * cat(state, x)[b, h, k] + bias[h]
  new_state[b, h, :] = cat(state, x)[b, h, 1:]

Sharding: batch dim (4096) split across 8 cores, 512 rows each; weights
replicated. Per core, batch rows sit on SBUF partitions and state is viewed
flat as [B, H*3] so every DMA is contiguous per partition.

Weights arrive as one concatenated row (per H-chunk [w3_c | taps interleaved],
then bias) and are broadcast across the 128 partitions on-chip with one-hot
PE matmuls (PSUM) evacuated just-in-time ahead of their consuming chunk.

Each H-chunk tile holds [x_c | state_c | spare]; one contiguous product pass
multiplies both the three taps and the x term by their weights, then four
adds reduce taps + x-term + bias. new_state needs no separate tile: the flat
new_state row is the state row shifted left by one with every 3rd slot
overwritten by x, so x is scattered into the state region (after the product
pass consumed it) and a shifted contiguous view is stored.
"""

import numpy as np

import concourse.bass as bass
import concourse.tile as tile
from concourse import bacc, mybir
from concourse.bass_utils import run_bass_kernel_spmd

B = 4096
H = 4096
K = 4
NCORES = 8
BS = B // NCORES          # 512 batch rows per core
P = 128                   # SBUF partitions
NBT = BS // P             # 4 batch tiles per core
HC = 1024                 # H chunk
NHC = H // HC             # 4 chunks
SC = 3 * HC               # state columns per chunk (3072)
CW = HC + SC              # per-chunk weight/product width (4096)
WTOT = NHC * CW + H       # wall columns: per-chunk blocks + bias (20480)
MMN = 512                 # PSUM free-dim per broadcast matmul
WR = 8                    # weight-row partitions
WC = WTOT // WR           # 2560 cols per weight row

_cache = {}


def _build_program():
    f32 = mybir.dt.float32
    nc = bacc.Bacc("TRN2", target_bir_lowering=False, debug=False)

    x_d = nc.dram_tensor("x", [BS, H], f32, kind="ExternalInput").ap()
    st_d = nc.dram_tensor("state", [BS, 3 * H], f32, kind="ExternalInput").ap()
    w_d = nc.dram_tensor("wrow", [1, WTOT], f32, kind="ExternalInput").ap()
    oh_d = nc.dram_tensor("onehot", [WR, WR * P], f32, kind="ExternalInput").ap()
    out_d = nc.dram_tensor("out", [BS, H], f32, kind="ExternalOutput").ap()
    ns_d = nc.dram_tensor("new_state", [BS, 3 * H], f32, kind="ExternalOutput").ap()

    with tile.TileContext(nc) as tc:
        with (
            tc.tile_pool(name="weights", bufs=1) as wpool,
            tc.tile_pool(name="wrows", bufs=1) as rpool,
            tc.tile_pool(name="psum", bufs=4, space="PSUM") as ppool,
            tc.tile_pool(name="oacc", bufs=3) as opool,
            tc.tile_pool(name="sdata", bufs=4) as spool,
            tc.tile_pool(name="tprod", bufs=1) as tpool,
        ):
            # one-hot selector: oh[k, 128*r + m] = (k == r)
            oh = rpool.tile([WR, WR * P], f32, tag="oh")
            nc.sync.dma_start(out=oh[:], in_=oh_d[:])
            wrow = rpool.tile([WR, WC], f32, tag="wrow")
            nc.sync.dma_start(
                out=wrow[:], in_=w_d.rearrange("o (a b) -> (o a) b", b=WC)
            )
            wall = wpool.tile([P, WTOT], f32, tag="wall")

            def bcast_unit(u):
                """Broadcast wall cols [1024*u, 1024*u+1024) from the weight rows."""
                pt = ppool.tile([P, 2 * MMN], f32, tag="pt")
                for i in range(2):
                    s = 2 * u + i
                    r, cblk = divmod(s, WC // MMN)
                    nc.tensor.matmul(
                        pt[:, i * MMN : (i + 1) * MMN],
                        oh[:, r * P : (r + 1) * P],
                        wrow[:, cblk * MMN : (cblk + 1) * MMN],
                        start=True,
                        stop=True,
                    )
                nc.scalar.copy(wall[:, 2 * u * MMN : 2 * (u + 1) * MMN], pt[:])

            def bcast_chunk(c):
                """Queue the broadcast of chunk c's weight block + bias slice."""
                for u in range(c * CW // HC, (c + 1) * CW // HC):
                    bcast_unit(u)
                bcast_unit((NHC * CW + c * HC) // HC)

            bcast_chunk(0)
            # chunk-outer: the first four iterations need only chunk-0's
            # weights; chunk c+1's weights are evacuated during chunk c's
            # first iteration (after the ns-store issue, before the
            # out-store's cross-engine wait can block the in-order ACT
            # queue), giving a full batch-sweep of lead time.
            for c in range(NHC):
                for bt in range(NBT):
                    r0 = bt * P
                    st = spool.tile([P, CW + 1], f32, tag="st")
                    nc.sync.dma_start(
                        out=st[:, 0:HC], in_=x_d[r0 : r0 + P, c * HC : (c + 1) * HC]
                    )
                    nc.sync.dma_start(
                        out=st[:, HC:CW], in_=st_d[r0 : r0 + P, c * SC : (c + 1) * SC]
                    )
                    tp = tpool.tile([P, CW], f32, tag="tp")
                    ot = opool.tile([P, HC], f32, tag="ot")
                    wc = c * CW
                    nc.vector.tensor_mul(tp[:], st[:, 0:CW], wall[:, wc : wc + CW])
                    nc.vector.tensor_add(ot[:], tp[:, HC:CW:3], tp[:, HC + 1 : CW : 3])
                    nc.vector.tensor_add(ot[:], ot[:], tp[:, HC + 2 : CW : 3])
                    # x-term and bias adds run on the otherwise-idle GpSimd
                    # engine, shortening the DVE critical path
                    nc.gpsimd.tensor_add(ot[:], ot[:], tp[:, 0:HC])
                    bc = NHC * CW + c * HC
                    nc.gpsimd.tensor_add(ot[:], ot[:], wall[:, bc : bc + HC])
                    # shift-register tail: x becomes newest tap of new_state
                    nc.scalar.copy(st[:, HC + 3 : CW + 1 : 3], st[:, 0:HC])
                    nc.scalar.dma_start(
                        out=ns_d[r0 : r0 + P, c * SC : (c + 1) * SC],
                        in_=st[:, HC + 1 : CW + 1],
                    )
                    if bt == 0 and c + 1 < NHC:
                        bcast_chunk(c + 1)
                    nc.scalar.dma_start(
                        out=out_d[r0 : r0 + P, c * HC : (c + 1) * HC], in_=ot[:]
                    )

    nc.compile()
    return nc


def _get_program():
    if "nc" not in _cache:
        _cache["nc"] = _build_program()
    return _cache["nc"]


def _pack_weights(weight, bias):
    w = np.asarray(weight, dtype=np.float32)
    row = np.empty(WTOT, dtype=np.float32)
    for c in range(NHC):
        h0 = c * HC
        row[c * CW : c * CW + HC] = w[h0 : h0 + HC, 3]
        row[c * CW + HC : (c + 1) * CW] = w[h0 : h0 + HC, 0:3].reshape(-1)
    row[NHC * CW :] = np.asarray(bias, dtype=np.float32)
    return row.reshape(1, WTOT)


def run(x, state, weight, bias, trace=False, **spmd_kwargs):
    nc = _get_program()

    state_f = np.ascontiguousarray(state, dtype=np.float32).reshape(B, 3 * H)
    x = np.ascontiguousarray(x, dtype=np.float32)
    wrow = _pack_weights(weight, bias)
    onehot = np.zeros((WR, WR * P), dtype=np.float32)
    for r in range(WR):
        onehot[r, r * P : (r + 1) * P] = 1.0

    in_maps = [
        {
            "x": x[i * BS : (i + 1) * BS],
            "state": state_f[i * BS : (i + 1) * BS],
            "wrow": wrow,
            "onehot": onehot,
        }
        for i in range(NCORES)
    ]
    res = run_bass_kernel_spmd(
        nc, in_maps, list(range(NCORES)), trace=trace, **spmd_kwargs
    )
    out = np.concatenate([res.results[i]["out"] for i in range(NCORES)], axis=0)
    new_state = np.concatenate(
        [res.results[i]["new_state"] for i in range(NCORES)], axis=0
    ).reshape(B, H, K - 1)
    return (out, new_state), res


def kernel(x, state, weight, bias):
    (out, new_state), _ = run(x, state, weight, bias, trace=False)
    return out, new_state


# revision 24
# speedup vs baseline: 1.0686x; 1.0686x over previous
"""Depthwise causal-conv1d step (single timestep) on 8 Trainium2 cores.

  out[b, h]         = sum_k w[h, k] * cat(state, x)[b, h, k] + bias[h]
  new_state[b, h, :] = cat(state, x)[b, h, 1:]

Sharding: batch dim (4096) split across 8 cores, 512 rows each; weights
replicated. Per core, batch rows sit on SBUF partitions and state is viewed
flat as [B, H*3] so every DMA is contiguous per partition.

Weights arrive as one concatenated row (per H-chunk [w3_c | taps interleaved],
then bias) and are broadcast across the 128 partitions on-chip with one-hot
PE matmuls (PSUM) evacuated just-in-time ahead of their consuming chunk.

Each H-chunk tile holds [x_c | state_c | spare]; one contiguous product pass
multiplies both the three taps and the x term by their weights, then four
adds reduce taps + x-term + bias. new_state needs no separate tile: the flat
new_state row is the state row shifted left by one with every 3rd slot
overwritten by x, so x is scattered into the state region (after the product
pass consumed it) and a shifted contiguous view is stored.
"""

import numpy as np

import concourse.bass as bass
import concourse.tile as tile
from concourse import bacc, mybir
from concourse.bass_utils import run_bass_kernel_spmd

B = 4096
H = 4096
K = 4
NCORES = 8
BS = B // NCORES          # 512 batch rows per core
P = 128                   # SBUF partitions
NBT = BS // P             # 4 batch tiles per core
HC = 1024                 # H chunk
NHC = H // HC             # 4 chunks
SC = 3 * HC               # state columns per chunk (3072)
CW = HC + SC              # per-chunk weight/product width (4096)
WTOT = NHC * CW + H       # wall columns: per-chunk blocks + bias (20480)
MMN = 512                 # PSUM free-dim per broadcast matmul
WR = 8                    # weight-row partitions
WC = WTOT // WR           # 2560 cols per weight row

_cache = {}


def _build_program():
    f32 = mybir.dt.float32
    nc = bacc.Bacc("TRN2", target_bir_lowering=False, debug=False)

    x_d = nc.dram_tensor("x", [BS, H], f32, kind="ExternalInput").ap()
    st_d = nc.dram_tensor("state", [BS, 3 * H], f32, kind="ExternalInput").ap()
    w_d = nc.dram_tensor("wrow", [1, WTOT], f32, kind="ExternalInput").ap()
    oh_d = nc.dram_tensor("onehot", [WR, WR * P], f32, kind="ExternalInput").ap()
    out_d = nc.dram_tensor("out", [BS, H], f32, kind="ExternalOutput").ap()
    ns_d = nc.dram_tensor("new_state", [BS, 3 * H], f32, kind="ExternalOutput").ap()

    with tile.TileContext(nc) as tc:
        with (
            tc.tile_pool(name="weights", bufs=1) as wpool,
            tc.tile_pool(name="wrows", bufs=1) as rpool,
            tc.tile_pool(name="psum", bufs=4, space="PSUM") as ppool,
            tc.tile_pool(name="oacc", bufs=3) as opool,
            tc.tile_pool(name="sdata", bufs=4) as spool,
            tc.tile_pool(name="tprod", bufs=1) as tpool,
        ):
            # one-hot selector: oh[k, 128*r + m] = (k == r)
            oh = rpool.tile([WR, WR * P], f32, tag="oh")
            nc.sync.dma_start(out=oh[:], in_=oh_d[:])
            wrow = rpool.tile([WR, WC], f32, tag="wrow")
            nc.sync.dma_start(
                out=wrow[:], in_=w_d.rearrange("o (a b) -> (o a) b", b=WC)
            )
            wall = wpool.tile([P, WTOT], f32, tag="wall")

            def bcast_unit(u):
                """Broadcast wall cols [1024*u, 1024*u+1024) from the weight rows."""
                pt = ppool.tile([P, 2 * MMN], f32, tag="pt")
                for i in range(2):
                    s = 2 * u + i
                    r, cblk = divmod(s, WC // MMN)
                    nc.tensor.matmul(
                        pt[:, i * MMN : (i + 1) * MMN],
                        oh[:, r * P : (r + 1) * P],
                        wrow[:, cblk * MMN : (cblk + 1) * MMN],
                        start=True,
                        stop=True,
                    )
                nc.scalar.copy(wall[:, 2 * u * MMN : 2 * (u + 1) * MMN], pt[:])

            def bcast_chunk(c):
                """Queue the broadcast of chunk c's weight block + bias slice."""
                for u in range(c * CW // HC, (c + 1) * CW // HC):
                    bcast_unit(u)
                bcast_unit((NHC * CW + c * HC) // HC)

            bcast_chunk(0)
            # chunk-outer: the first four iterations need only chunk-0's
            # weights; chunk c+1's weights are evacuated during chunk c's
            # first iteration (after the ns-store issue, before the
            # out-store's cross-engine wait can block the in-order ACT
            # queue), giving a full batch-sweep of lead time.
            for c in range(NHC):
                for bt in range(NBT):
                    r0 = bt * P
                    # layout: [x (1024) | pad (7) | state (3072) | spare]
                    # state at S0=1031 puts the shifted new_state view at
                    # col 1032 -> 32B-aligned DMA reads for the ns store
                    S0 = HC + 7
                    st = spool.tile([P, S0 + SC + 1], f32, tag="st")
                    nc.sync.dma_start(
                        out=st[:, 0:HC], in_=x_d[r0 : r0 + P, c * HC : (c + 1) * HC]
                    )
                    nc.sync.dma_start(
                        out=st[:, S0 : S0 + SC],
                        in_=st_d[r0 : r0 + P, c * SC : (c + 1) * SC],
                    )
                    tp = tpool.tile([P, CW], f32, tag="tp")
                    ot = opool.tile([P, HC], f32, tag="ot")
                    wc = c * CW
                    # tap products written deinterleaved (tp[k*HC + h]) so the
                    # reduction adds below are all contiguous
                    nc.vector.tensor_mul(
                        tp[:, 0:SC].rearrange("p (k h) -> p h k", h=HC),
                        st[:, S0 : S0 + SC].rearrange("p (h k) -> p h k", k=3),
                        wall[:, wc + HC : wc + CW].rearrange("p (h k) -> p h k", k=3),
                    )
                    # shift-register tail: x becomes newest tap of new_state
                    nc.scalar.copy(st[:, S0 + 3 : S0 + SC + 1 : 3], st[:, 0:HC])
                    nc.vector.tensor_mul(
                        tp[:, SC:CW], st[:, 0:HC], wall[:, wc : wc + HC]
                    )
                    nc.vector.tensor_add(ot[:], tp[:, 0:HC], tp[:, HC : 2 * HC])
                    nc.vector.tensor_add(ot[:], ot[:], tp[:, 2 * HC : SC])
                    nc.vector.tensor_add(ot[:], ot[:], tp[:, SC:CW])
                    bc = NHC * CW + c * HC
                    nc.vector.tensor_add(ot[:], ot[:], wall[:, bc : bc + HC])
                    nc.scalar.dma_start(
                        out=ns_d[r0 : r0 + P, c * SC : (c + 1) * SC],
                        in_=st[:, S0 + 1 : S0 + SC + 1],
                    )
                    if bt == 0 and c + 1 < NHC:
                        bcast_chunk(c + 1)
                    nc.scalar.dma_start(
                        out=out_d[r0 : r0 + P, c * HC : (c + 1) * HC], in_=ot[:]
                    )

    nc.compile()
    return nc


def _get_program():
    if "nc" not in _cache:
        _cache["nc"] = _build_program()
    return _cache["nc"]


def _pack_weights(weight, bias):
    w = np.asarray(weight, dtype=np.float32)
    row = np.empty(WTOT, dtype=np.float32)
    for c in range(NHC):
        h0 = c * HC
        row[c * CW : c * CW + HC] = w[h0 : h0 + HC, 3]
        row[c * CW + HC : (c + 1) * CW] = w[h0 : h0 + HC, 0:3].reshape(-1)
    row[NHC * CW :] = np.asarray(bias, dtype=np.float32)
    return row.reshape(1, WTOT)


def run(x, state, weight, bias, trace=False, **spmd_kwargs):
    nc = _get_program()

    state_f = np.ascontiguousarray(state, dtype=np.float32).reshape(B, 3 * H)
    x = np.ascontiguousarray(x, dtype=np.float32)
    wrow = _pack_weights(weight, bias)
    onehot = np.zeros((WR, WR * P), dtype=np.float32)
    for r in range(WR):
        onehot[r, r * P : (r + 1) * P] = 1.0

    in_maps = [
        {
            "x": x[i * BS : (i + 1) * BS],
            "state": state_f[i * BS : (i + 1) * BS],
            "wrow": wrow,
            "onehot": onehot,
        }
        for i in range(NCORES)
    ]
    res = run_bass_kernel_spmd(
        nc, in_maps, list(range(NCORES)), trace=trace, **spmd_kwargs
    )
    out = np.concatenate([res.results[i]["out"] for i in range(NCORES)], axis=0)
    new_state = np.concatenate(
        [res.results[i]["new_state"] for i in range(NCORES)], axis=0
    ).reshape(B, H, K - 1)
    return (out, new_state), res


def kernel(x, state, weight, bias):
    (out, new_state), _ = run(x, state, weight, bias, trace=False)
    return out, new_state


# revision 30
# speedup vs baseline: 1.0813x; 1.0119x over previous
"""Depthwise causal-conv1d step (single timestep) on 8 Trainium2 cores.

  out[b, h]         = sum_k w[h, k] * cat(state, x)[b, h, k] + bias[h]
  new_state[b, h, :] = cat(state, x)[b, h, 1:]

Sharding: batch dim (4096) split across 8 cores, 512 rows each; weights
replicated. Per core, batch rows sit on SBUF partitions and state is viewed
flat as [B, H*3] so every DMA is contiguous per partition.

Weights arrive as one concatenated row (per H-chunk [w3_c | taps interleaved],
then bias) and are broadcast across the 128 partitions on-chip with one-hot
PE matmuls (PSUM) evacuated just-in-time ahead of their consuming chunk.

Each H-chunk tile holds [x_c | state_c | spare]; one contiguous product pass
multiplies both the three taps and the x term by their weights, then four
adds reduce taps + x-term + bias. new_state needs no separate tile: the flat
new_state row is the state row shifted left by one with every 3rd slot
overwritten by x, so x is scattered into the state region (after the product
pass consumed it) and a shifted contiguous view is stored.
"""

import numpy as np

import concourse.bass as bass
import concourse.tile as tile
from concourse import bacc, mybir
from concourse.bass_utils import run_bass_kernel_spmd

B = 4096
H = 4096
K = 4
NCORES = 8
BS = B // NCORES          # 512 batch rows per core
P = 128                   # SBUF partitions
NBT = BS // P             # 4 batch tiles per core
HC = 1024                 # H chunk
NHC = H // HC             # 4 chunks
SC = 3 * HC               # state columns per chunk (3072)
CW = HC + SC              # per-chunk weight/product width (4096)
WTOT = NHC * CW + H       # wall columns: per-chunk blocks + bias (20480)
MMN = 512                 # PSUM free-dim per broadcast matmul
WR = 8                    # weight-row partitions
WC = WTOT // WR           # 2560 cols per weight row

_cache = {}


def _build_program():
    f32 = mybir.dt.float32
    nc = bacc.Bacc("TRN2", target_bir_lowering=False, debug=False)

    x_d = nc.dram_tensor("x", [BS, H], f32, kind="ExternalInput").ap()
    st_d = nc.dram_tensor("state", [BS, 3 * H], f32, kind="ExternalInput").ap()
    w_d = nc.dram_tensor("wrow", [1, WTOT], f32, kind="ExternalInput").ap()
    oh_d = nc.dram_tensor("onehot", [WR, WR * P], f32, kind="ExternalInput").ap()
    # chunk-0 weights pre-broadcast on the host: loaded straight into wall so
    # the first compute never waits on the PE broadcast chain
    w0_d = nc.dram_tensor("w0b", [P, CW + HC], f32, kind="ExternalInput").ap()
    out_d = nc.dram_tensor("out", [BS, H], f32, kind="ExternalOutput").ap()
    ns_d = nc.dram_tensor("new_state", [BS, 3 * H], f32, kind="ExternalOutput").ap()

    with tile.TileContext(nc) as tc:
        with (
            tc.tile_pool(name="weights", bufs=1) as wpool,
            tc.tile_pool(name="wrows", bufs=1) as rpool,
            tc.tile_pool(name="psum", bufs=4, space="PSUM") as ppool,
            tc.tile_pool(name="oacc", bufs=3) as opool,
            tc.tile_pool(name="sdata", bufs=4) as spool,
            tc.tile_pool(name="tprod", bufs=1) as tpool,
        ):
            # one-hot selector: oh[k, 128*r + m] = (k == r)
            oh = rpool.tile([WR, WR * P], f32, tag="oh")
            nc.sync.dma_start(out=oh[:], in_=oh_d[:])
            wrow = rpool.tile([WR, WC], f32, tag="wrow")
            nc.sync.dma_start(
                out=wrow[:], in_=w_d.rearrange("o (a b) -> (o a) b", b=WC)
            )
            wall = wpool.tile([P, WTOT], f32, tag="wall")

            def bcast_unit(u):
                """Broadcast wall cols [1024*u, 1024*u+1024) from the weight rows."""
                pt = ppool.tile([P, 2 * MMN], f32, tag="pt")
                for i in range(2):
                    s = 2 * u + i
                    r, cblk = divmod(s, WC // MMN)
                    nc.tensor.matmul(
                        pt[:, i * MMN : (i + 1) * MMN],
                        oh[:, r * P : (r + 1) * P],
                        wrow[:, cblk * MMN : (cblk + 1) * MMN],
                        start=True,
                        stop=True,
                    )
                nc.scalar.copy(wall[:, 2 * u * MMN : 2 * (u + 1) * MMN], pt[:])

            def bcast_chunk(c):
                """Queue the broadcast of chunk c's weight block + bias slice."""
                for u in range(c * CW // HC, (c + 1) * CW // HC):
                    bcast_unit(u)
                bcast_unit((NHC * CW + c * HC) // HC)

            nc.sync.dma_start(out=wall[:, 0:CW], in_=w0_d[:, 0:CW])
            nc.sync.dma_start(
                out=wall[:, NHC * CW : NHC * CW + HC], in_=w0_d[:, CW : CW + HC]
            )
            bcast_chunk(1)
            # chunk-outer: the first four iterations need only chunk-0's
            # weights (DMA'd pre-broadcast above); chunk c+2's weights are
            # evacuated during chunk c's first iteration (after the ns-store
            # issue, before the out-store's cross-engine wait can block the
            # in-order ACT queue), giving over a batch-sweep of lead time.
            for c in range(NHC):
                for bt in range(NBT):
                    r0 = bt * P
                    st = spool.tile([P, CW + 1], f32, tag="st")
                    nc.sync.dma_start(
                        out=st[:, 0:HC], in_=x_d[r0 : r0 + P, c * HC : (c + 1) * HC]
                    )
                    nc.sync.dma_start(
                        out=st[:, HC:CW], in_=st_d[r0 : r0 + P, c * SC : (c + 1) * SC]
                    )
                    tp = tpool.tile([P, CW], f32, tag="tp")
                    ot = opool.tile([P, HC], f32, tag="ot")
                    wc = c * CW
                    nc.vector.tensor_mul(tp[:], st[:, 0:CW], wall[:, wc : wc + CW])
                    nc.vector.tensor_add(ot[:], tp[:, HC:CW:3], tp[:, HC + 1 : CW : 3])
                    nc.vector.tensor_add(ot[:], ot[:], tp[:, HC + 2 : CW : 3])
                    nc.vector.tensor_add(ot[:], ot[:], tp[:, 0:HC])
                    bc = NHC * CW + c * HC
                    nc.vector.tensor_add(ot[:], ot[:], wall[:, bc : bc + HC])
                    # shift-register tail: x becomes newest tap of new_state
                    nc.scalar.copy(st[:, HC + 3 : CW + 1 : 3], st[:, 0:HC])
                    nc.scalar.dma_start(
                        out=ns_d[r0 : r0 + P, c * SC : (c + 1) * SC],
                        in_=st[:, HC + 1 : CW + 1],
                    )
                    if bt == 0 and c + 2 < NHC:
                        bcast_chunk(c + 2)
                    nc.scalar.dma_start(
                        out=out_d[r0 : r0 + P, c * HC : (c + 1) * HC], in_=ot[:]
                    )

    nc.compile()
    return nc


def _get_program():
    if "nc" not in _cache:
        _cache["nc"] = _build_program()
    return _cache["nc"]


def _pack_weights(weight, bias):
    w = np.asarray(weight, dtype=np.float32)
    row = np.empty(WTOT, dtype=np.float32)
    for c in range(NHC):
        h0 = c * HC
        row[c * CW : c * CW + HC] = w[h0 : h0 + HC, 3]
        row[c * CW + HC : (c + 1) * CW] = w[h0 : h0 + HC, 0:3].reshape(-1)
    row[NHC * CW :] = np.asarray(bias, dtype=np.float32)
    return row.reshape(1, WTOT)


def run(x, state, weight, bias, trace=False, **spmd_kwargs):
    nc = _get_program()

    state_f = np.ascontiguousarray(state, dtype=np.float32).reshape(B, 3 * H)
    x = np.ascontiguousarray(x, dtype=np.float32)
    wrow = _pack_weights(weight, bias)
    onehot = np.zeros((WR, WR * P), dtype=np.float32)
    for r in range(WR):
        onehot[r, r * P : (r + 1) * P] = 1.0
    w0row = np.concatenate([wrow[0, 0:CW], wrow[0, NHC * CW : NHC * CW + HC]])
    w0b = np.ascontiguousarray(np.broadcast_to(w0row[None, :], (P, CW + HC)))

    in_maps = [
        {
            "x": x[i * BS : (i + 1) * BS],
            "state": state_f[i * BS : (i + 1) * BS],
            "wrow": wrow,
            "onehot": onehot,
            "w0b": w0b,
        }
        for i in range(NCORES)
    ]
    res = run_bass_kernel_spmd(
        nc, in_maps, list(range(NCORES)), trace=trace, **spmd_kwargs
    )
    out = np.concatenate([res.results[i]["out"] for i in range(NCORES)], axis=0)
    new_state = np.concatenate(
        [res.results[i]["new_state"] for i in range(NCORES)], axis=0
    ).reshape(B, H, K - 1)
    return (out, new_state), res


def kernel(x, state, weight, bias):
    (out, new_state), _ = run(x, state, weight, bias, trace=False)
    return out, new_state


# revision 36
# speedup vs baseline: 1.0915x; 1.0094x over previous
"""Depthwise causal-conv1d step (single timestep) on 8 Trainium2 cores.

  out[b, h]         = sum_k w[h, k] * cat(state, x)[b, h, k] + bias[h]
  new_state[b, h, :] = cat(state, x)[b, h, 1:]

Sharding: batch dim (4096) split across 8 cores, 512 rows each; weights
replicated. Per core, batch rows sit on SBUF partitions and state is viewed
flat as [B, H*3] so every DMA is contiguous per partition.

Weights arrive as one concatenated row (per H-chunk [w3_c | taps interleaved],
then bias) and are broadcast across the 128 partitions on-chip with one-hot
PE matmuls (PSUM) evacuated just-in-time ahead of their consuming chunk.

Each H-chunk tile holds [x_c | state_c | spare]; one contiguous product pass
multiplies both the three taps and the x term by their weights, then four
adds reduce taps + x-term + bias. new_state needs no separate tile: the flat
new_state row is the state row shifted left by one with every 3rd slot
overwritten by x, so x is scattered into the state region (after the product
pass consumed it) and a shifted contiguous view is stored.
"""

import numpy as np

import concourse.bass as bass
import concourse.tile as tile
from concourse import bacc, mybir
from concourse.bass_utils import run_bass_kernel_spmd

B = 4096
H = 4096
K = 4
NCORES = 8
BS = B // NCORES          # 512 batch rows per core
P = 128                   # SBUF partitions
NBT = BS // P             # 4 batch tiles per core
HC = 1024                 # H chunk
NHC = H // HC             # 4 chunks
SC = 3 * HC               # state columns per chunk (3072)
CW = HC + SC              # per-chunk weight/product width (4096)
WTOT = NHC * CW + H       # wall columns: per-chunk blocks + bias (20480)
MMN = 512                 # PSUM free-dim per broadcast matmul
WR = 8                    # weight-row partitions
WC = WTOT // WR           # 2560 cols per weight row

_cache = {}


def _build_program():
    f32 = mybir.dt.float32
    nc = bacc.Bacc("TRN2", target_bir_lowering=False, debug=False)

    x_d = nc.dram_tensor("x", [BS, H], f32, kind="ExternalInput").ap()
    st_d = nc.dram_tensor("state", [BS, 3 * H], f32, kind="ExternalInput").ap()
    w_d = nc.dram_tensor("wrow", [1, WTOT], f32, kind="ExternalInput").ap()
    oh_d = nc.dram_tensor("onehot", [WR, WR * P], f32, kind="ExternalInput").ap()
    out_d = nc.dram_tensor("out", [BS, H], f32, kind="ExternalOutput").ap()
    ns_d = nc.dram_tensor("new_state", [BS, 3 * H], f32, kind="ExternalOutput").ap()

    with tile.TileContext(nc) as tc:
        with (
            tc.tile_pool(name="weights", bufs=1) as wpool,
            tc.tile_pool(name="wrows", bufs=1) as rpool,
            tc.tile_pool(name="psum", bufs=4, space="PSUM") as ppool,
            tc.tile_pool(name="oacc", bufs=3) as opool,
            tc.tile_pool(name="sdata", bufs=4) as spool,
            tc.tile_pool(name="tprod", bufs=1) as tpool,
        ):
            # one-hot selector: oh[k, 128*r + m] = (k == r)
            oh = rpool.tile([WR, WR * P], f32, tag="oh")
            nc.sync.dma_start(out=oh[:], in_=oh_d[:])
            wrow = rpool.tile([WR, WC], f32, tag="wrow")
            nc.sync.dma_start(
                out=wrow[:], in_=w_d.rearrange("o (a b) -> (o a) b", b=WC)
            )
            wall = wpool.tile([P, WTOT], f32, tag="wall")

            def bcast_unit(u, ev):
                """Broadcast wall cols [1024*u, 1024*u+1024) from the weight rows."""
                pt = ppool.tile([P, 2 * MMN], f32, tag="pt")
                for i in range(2):
                    s = 2 * u + i
                    r, cblk = divmod(s, WC // MMN)
                    nc.tensor.matmul(
                        pt[:, i * MMN : (i + 1) * MMN],
                        oh[:, r * P : (r + 1) * P],
                        wrow[:, cblk * MMN : (cblk + 1) * MMN],
                        start=True,
                        stop=True,
                    )
                dst = wall[:, 2 * u * MMN : 2 * (u + 1) * MMN]
                if ev is nc.vector:
                    nc.vector.tensor_copy(dst, pt[:])
                else:
                    nc.scalar.copy(dst, pt[:])

            def bcast_chunk(c, ev=None):
                """Queue the broadcast of chunk c's weight block + bias slice."""
                ev = ev or nc.scalar
                for u in range(c * CW // HC, (c + 1) * CW // HC):
                    bcast_unit(u, ev)
                bcast_unit((NHC * CW + c * HC) // HC, ev)

            # chunk-0 units evacuated on the Vector engine (idle before the
            # first product pass, and DVE PSUM copies dodge the ~3.4us/unit
            # PE->ACT handshake that would gate the first compute)
            bcast_chunk(0, ev=nc.vector)
            # chunk-outer: the first four iterations need only chunk-0's
            # weights; chunk c+1's weights are evacuated during chunk c's
            # first iteration (after the ns-store issue, before the
            # out-store's cross-engine wait can block the in-order ACT
            # queue), giving a full batch-sweep of lead time.
            for c in range(NHC):
                for bt in range(NBT):
                    r0 = bt * P
                    st = spool.tile([P, CW + 1], f32, tag="st")
                    nc.sync.dma_start(
                        out=st[:, 0:HC], in_=x_d[r0 : r0 + P, c * HC : (c + 1) * HC]
                    )
                    nc.sync.dma_start(
                        out=st[:, HC:CW], in_=st_d[r0 : r0 + P, c * SC : (c + 1) * SC]
                    )
                    tp = tpool.tile([P, CW], f32, tag="tp")
                    ot = opool.tile([P, HC], f32, tag="ot")
                    wc = c * CW
                    nc.vector.tensor_mul(tp[:], st[:, 0:CW], wall[:, wc : wc + CW])
                    nc.vector.tensor_add(ot[:], tp[:, HC:CW:3], tp[:, HC + 1 : CW : 3])
                    nc.vector.tensor_add(ot[:], ot[:], tp[:, HC + 2 : CW : 3])
                    nc.vector.tensor_add(ot[:], ot[:], tp[:, 0:HC])
                    bc = NHC * CW + c * HC
                    nc.vector.tensor_add(ot[:], ot[:], wall[:, bc : bc + HC])
                    # shift-register tail: x becomes newest tap of new_state
                    nc.scalar.copy(st[:, HC + 3 : CW + 1 : 3], st[:, 0:HC])
                    nc.scalar.dma_start(
                        out=ns_d[r0 : r0 + P, c * SC : (c + 1) * SC],
                        in_=st[:, HC + 1 : CW + 1],
                    )
                    if bt == 0 and c + 1 < NHC:
                        bcast_chunk(c + 1)
                    nc.scalar.dma_start(
                        out=out_d[r0 : r0 + P, c * HC : (c + 1) * HC], in_=ot[:]
                    )

    nc.compile()
    return nc


def _get_program():
    if "nc" not in _cache:
        _cache["nc"] = _build_program()
    return _cache["nc"]


def _pack_weights(weight, bias):
    w = np.asarray(weight, dtype=np.float32)
    row = np.empty(WTOT, dtype=np.float32)
    for c in range(NHC):
        h0 = c * HC
        row[c * CW : c * CW + HC] = w[h0 : h0 + HC, 3]
        row[c * CW + HC : (c + 1) * CW] = w[h0 : h0 + HC, 0:3].reshape(-1)
    row[NHC * CW :] = np.asarray(bias, dtype=np.float32)
    return row.reshape(1, WTOT)


def run(x, state, weight, bias, trace=False, **spmd_kwargs):
    nc = _get_program()

    state_f = np.ascontiguousarray(state, dtype=np.float32).reshape(B, 3 * H)
    x = np.ascontiguousarray(x, dtype=np.float32)
    wrow = _pack_weights(weight, bias)
    onehot = np.zeros((WR, WR * P), dtype=np.float32)
    for r in range(WR):
        onehot[r, r * P : (r + 1) * P] = 1.0

    in_maps = [
        {
            "x": x[i * BS : (i + 1) * BS],
            "state": state_f[i * BS : (i + 1) * BS],
            "wrow": wrow,
            "onehot": onehot,
        }
        for i in range(NCORES)
    ]
    res = run_bass_kernel_spmd(
        nc, in_maps, list(range(NCORES)), trace=trace, **spmd_kwargs
    )
    out = np.concatenate([res.results[i]["out"] for i in range(NCORES)], axis=0)
    new_state = np.concatenate(
        [res.results[i]["new_state"] for i in range(NCORES)], axis=0
    ).reshape(B, H, K - 1)
    return (out, new_state), res


def kernel(x, state, weight, bias):
    (out, new_state), _ = run(x, state, weight, bias, trace=False)
    return out, new_state


# revision 37
# speedup vs baseline: 1.1908x; 1.0910x over previous
"""Depthwise causal-conv1d step (single timestep) on 8 Trainium2 cores.

  out[b, h]         = sum_k w[h, k] * cat(state, x)[b, h, k] + bias[h]
  new_state[b, h, :] = cat(state, x)[b, h, 1:]

Sharding: batch dim (4096) split across 8 cores, 512 rows each; weights
replicated. Per core, batch rows sit on SBUF partitions and state is viewed
flat as [B, H*3] so every DMA is contiguous per partition.

Weights arrive as one concatenated row (per H-chunk [w3_c | taps interleaved],
then bias) and are broadcast across the 128 partitions on-chip with one-hot
PE matmuls (PSUM) evacuated just-in-time ahead of their consuming chunk.

Each H-chunk tile holds [x_c | state_c | spare]; one contiguous product pass
multiplies both the three taps and the x term by their weights, then four
adds reduce taps + x-term + bias. new_state needs no separate tile: the flat
new_state row is the state row shifted left by one with every 3rd slot
overwritten by x, so x is scattered into the state region (after the product
pass consumed it) and a shifted contiguous view is stored.
"""

import numpy as np

import concourse.bass as bass
import concourse.tile as tile
from concourse import bacc, mybir
from concourse.bass_utils import run_bass_kernel_spmd

B = 4096
H = 4096
K = 4
NCORES = 8
BS = B // NCORES          # 512 batch rows per core
P = 128                   # SBUF partitions
NBT = BS // P             # 4 batch tiles per core
HC = 1024                 # H chunk
NHC = H // HC             # 4 chunks
SC = 3 * HC               # state columns per chunk (3072)
CW = HC + SC              # per-chunk weight/product width (4096)
WTOT = NHC * CW + H       # wall columns: per-chunk blocks + bias (20480)
MMN = 512                 # PSUM free-dim per broadcast matmul
WR = 8                    # weight-row partitions
WC = WTOT // WR           # 2560 cols per weight row

_cache = {}


def _build_program():
    f32 = mybir.dt.float32
    nc = bacc.Bacc("TRN2", target_bir_lowering=False, debug=False)

    x_d = nc.dram_tensor("x", [BS, H], f32, kind="ExternalInput").ap()
    st_d = nc.dram_tensor("state", [BS, 3 * H], f32, kind="ExternalInput").ap()
    w_d = nc.dram_tensor("wrow", [1, WTOT], f32, kind="ExternalInput").ap()
    oh_d = nc.dram_tensor("onehot", [WR, WR * P], f32, kind="ExternalInput").ap()
    out_d = nc.dram_tensor("out", [BS, H], f32, kind="ExternalOutput").ap()
    ns_d = nc.dram_tensor("new_state", [BS, 3 * H], f32, kind="ExternalOutput").ap()

    with tile.TileContext(nc) as tc:
        with (
            tc.tile_pool(name="weights", bufs=1) as wpool,
            tc.tile_pool(name="wrows", bufs=1) as rpool,
            tc.tile_pool(name="psum", bufs=4, space="PSUM") as ppool,
            tc.tile_pool(name="oacc", bufs=3) as opool,
            tc.tile_pool(name="sdata", bufs=4) as spool,
            tc.tile_pool(name="tprod", bufs=1) as tpool,
        ):
            # one-hot selector: oh[k, 128*r + m] = (k == r)
            oh = rpool.tile([WR, WR * P], f32, tag="oh")
            nc.sync.dma_start(out=oh[:], in_=oh_d[:])
            wrow = rpool.tile([WR, WC], f32, tag="wrow")
            nc.sync.dma_start(
                out=wrow[:], in_=w_d.rearrange("o (a b) -> (o a) b", b=WC)
            )
            wall = wpool.tile([P, WTOT], f32, tag="wall")

            def bcast_unit(u, ev):
                """Broadcast wall cols [1024*u, 1024*u+1024) from the weight rows."""
                pt = ppool.tile([P, 2 * MMN], f32, tag="pt")
                for i in range(2):
                    s = 2 * u + i
                    r, cblk = divmod(s, WC // MMN)
                    nc.tensor.matmul(
                        pt[:, i * MMN : (i + 1) * MMN],
                        oh[:, r * P : (r + 1) * P],
                        wrow[:, cblk * MMN : (cblk + 1) * MMN],
                        start=True,
                        stop=True,
                    )
                dst = wall[:, 2 * u * MMN : 2 * (u + 1) * MMN]
                if ev is nc.vector:
                    nc.vector.tensor_copy(dst, pt[:])
                else:
                    nc.scalar.copy(dst, pt[:])

            def bcast_chunk(c, ev=None):
                """Queue the broadcast of chunk c's weight block + bias slice."""
                ev = ev or nc.scalar
                for u in range(c * CW // HC, (c + 1) * CW // HC):
                    bcast_unit(u, ev)
                bcast_unit((NHC * CW + c * HC) // HC, ev)

            bcast_chunk(0)
            # chunk-outer: the first four iterations need only chunk-0's
            # weights; chunk c+1's weights are evacuated during chunk c's
            # first iteration (after the ns-store issue, before the
            # out-store's cross-engine wait can block the in-order ACT
            # queue), giving a full batch-sweep of lead time.
            for c in range(NHC):
                for bt in range(NBT):
                    r0 = bt * P
                    st = spool.tile([P, CW + 1], f32, tag="st")
                    nc.sync.dma_start(
                        out=st[:, 0:HC], in_=x_d[r0 : r0 + P, c * HC : (c + 1) * HC]
                    )
                    nc.sync.dma_start(
                        out=st[:, HC:CW], in_=st_d[r0 : r0 + P, c * SC : (c + 1) * SC]
                    )
                    tp = tpool.tile([P, CW], f32, tag="tp")
                    ot = opool.tile([P, HC], f32, tag="ot")
                    wc = c * CW
                    nc.vector.tensor_mul(tp[:], st[:, 0:CW], wall[:, wc : wc + CW])
                    nc.vector.tensor_add(ot[:], tp[:, HC:CW:3], tp[:, HC + 1 : CW : 3])
                    nc.vector.tensor_add(ot[:], ot[:], tp[:, HC + 2 : CW : 3])
                    nc.vector.tensor_add(ot[:], ot[:], tp[:, 0:HC])
                    bc = NHC * CW + c * HC
                    nc.vector.tensor_add(ot[:], ot[:], wall[:, bc : bc + HC])
                    # shift-register tail: x becomes newest tap of new_state
                    nc.scalar.copy(st[:, HC + 3 : CW + 1 : 3], st[:, 0:HC])
                    nc.scalar.dma_start(
                        out=ns_d[r0 : r0 + P, c * SC : (c + 1) * SC],
                        in_=st[:, HC + 1 : CW + 1],
                    )
                    if bt == 0 and c + 1 < NHC:
                        bcast_chunk(c + 1)
                    nc.scalar.dma_start(
                        out=out_d[r0 : r0 + P, c * HC : (c + 1) * HC], in_=ot[:]
                    )

    nc.compile()
    return nc


def _get_program():
    if "nc" not in _cache:
        _cache["nc"] = _build_program()
    return _cache["nc"]


def _pack_weights(weight, bias):
    w = np.asarray(weight, dtype=np.float32)
    row = np.empty(WTOT, dtype=np.float32)
    for c in range(NHC):
        h0 = c * HC
        row[c * CW : c * CW + HC] = w[h0 : h0 + HC, 3]
        row[c * CW + HC : (c + 1) * CW] = w[h0 : h0 + HC, 0:3].reshape(-1)
    row[NHC * CW :] = np.asarray(bias, dtype=np.float32)
    return row.reshape(1, WTOT)


def run(x, state, weight, bias, trace=False, **spmd_kwargs):
    nc = _get_program()

    state_f = np.ascontiguousarray(state, dtype=np.float32).reshape(B, 3 * H)
    x = np.ascontiguousarray(x, dtype=np.float32)
    wrow = _pack_weights(weight, bias)
    onehot = np.zeros((WR, WR * P), dtype=np.float32)
    for r in range(WR):
        onehot[r, r * P : (r + 1) * P] = 1.0

    in_maps = [
        {
            "x": x[i * BS : (i + 1) * BS],
            "state": state_f[i * BS : (i + 1) * BS],
            "wrow": wrow,
            "onehot": onehot,
        }
        for i in range(NCORES)
    ]
    res = run_bass_kernel_spmd(
        nc, in_maps, list(range(NCORES)), trace=trace, **spmd_kwargs
    )
    out = np.concatenate([res.results[i]["out"] for i in range(NCORES)], axis=0)
    new_state = np.concatenate(
        [res.results[i]["new_state"] for i in range(NCORES)], axis=0
    ).reshape(B, H, K - 1)
    return (out, new_state), res


def kernel(x, state, weight, bias):
    (out, new_state), _ = run(x, state, weight, bias, trace=False)
    return out, new_state
